# revision 12
# baseline (speedup 1.0000x reference)
"""Trainium2 Bass kernel for nn_CrossAttention_56092272886201.

Talking-heads cross-attention, b=2, n=m=2048, dim=64, heads=8, dh=dv=8.
Sharding: 8 cores = (batch 2) x (query-chunks of 512). Each core is fully
independent (talking-heads mixes the heads axis, which stays on-core; the
query axis i is sharded), so no collectives are needed.

Per-core layout (i-tile = 16 queries x 8 heads = 128 partitions):
  S[(h,i), j]   = QK^T via block-diagonal packed q (one matmul, K=64)
  E = exp(S)    ACT, fused row-sum -> softmax denominator
  TW            = WD * (1/denom) per partition  (denom + talking-heads fold)
  A^T[j,(g,i)]  = talk matmul, lhsT=E-chunk (output already j-partitioned)
  m2            = sum_g A^2 (squares split ACT/DVE, strided tree adds on GPSIMD)
  r             = exp(-0.5*ln(m2/8+eps))   (rsqrt; ln+exp share one ACT table set)
  P = A*r       broadcast multiply
  out           = P @ V_ln (gamma_t folded into V, beta_t via K=1 matmul)
W_talk is centered host-side over g so the heads-LayerNorm mean is exactly 0.
"""

import numpy as np

DIM = 64
HEADS = 8
N = 2048
B = 2
NCORES = 8
ICHUNK = 512          # queries per core
NT = 32               # i-tiles per core (16 queries each)
EPS = 1e-5

_CACHE = {}


def _build(use_beta, use_gamma):
    import concourse.bacc as bacc
    import concourse.tile as tile
    from concourse import mybir

    F32 = mybir.dt.float32
    AX = mybir.AxisListType.X
    OP = mybir.AluOpType
    AF = mybir.ActivationFunctionType

    nc = bacc.Bacc()
    d_xT = nc.declare_dram_parameter("xT", [64, ICHUNK], F32, isOutput=False)
    d_ctxT = nc.declare_dram_parameter("ctxT", [64, N], F32, isOutput=False)
    d_wqT = nc.declare_dram_parameter("wqT", [64, 64], F32, isOutput=False)
    d_wkT = nc.declare_dram_parameter("wkT", [64, 64], F32, isOutput=False)
    d_wvT = nc.declare_dram_parameter("wvT", [64, 64], F32, isOutput=False)
    d_WD = nc.declare_dram_parameter("WD", [128, 128], F32, isOutput=False)
    d_beta = nc.declare_dram_parameter("beta", [1, 128], F32, isOutput=False)
    d_gtf = nc.declare_dram_parameter("gtf", [1, 64], F32, isOutput=False)
    d_gvf = nc.declare_dram_parameter("gvf", [1, 64], F32, isOutput=False)
    d_bvf = nc.declare_dram_parameter("bvf", [1, 64], F32, isOutput=False)
    d_mask = nc.declare_dram_parameter("mask", [1, 64], F32, isOutput=False)
    d_ones = nc.declare_dram_parameter("ones", [128, 1], F32, isOutput=False)
    d_out = nc.declare_dram_parameter("out", [NT * 128, 8], F32, isOutput=True)

    import concourse.bass as bass

    def bcast_ap(ap, levels):
        return bass.AP(tensor=ap.tensor, offset=ap.offset, ap=levels)

    with tile.TileContext(nc) as tc:
        with tc.tile_pool(name="statics", bufs=1) as st:
            xT = st.tile([64, ICHUNK], F32)
            ctxT = st.tile([64, N], F32)
            wqT = st.tile([64, 64], F32)
            wkT = st.tile([64, 64], F32)
            wvT = st.tile([64, 64], F32)
            WD = st.tile([128, 128], F32)
            beta = st.tile([1, 128], F32)
            gtf = st.tile([128, 64], F32)
            gvf = st.tile([128, 64], F32)
            bvf = st.tile([128, 64], F32)
            # head-select mask replicated to all partitions via DMA broadcast
            mask = st.tile([128, 64], F32)
            ones = st.tile([128, 1], F32)
            for sb, dr in ((xT, d_xT), (ctxT, d_ctxT), (wqT, d_wqT),
                           (wkT, d_wkT), (wvT, d_wvT), (WD, d_WD),
                           (beta, d_beta), (ones, d_ones)):
                nc.sync.dma_start(out=sb[:], in_=dr[:])
            # replicate [1,64] host rows across all 128 partitions
            for sb, dr in ((gtf, d_gtf), (gvf, d_gvf), (bvf, d_bvf)):
                nc.sync.dma_start(
                    out=sb[:], in_=bcast_ap(dr[:], [[0, 128], [1, 64]]))
            # mask[(g,i),(h,d)] = (h==g): zero then fill 8 diagonal blocks
            # from the [1,64] host ones-row via broadcast DMAs.
            mrow = d_mask.rearrange("o (h d) -> o h d", h=8)
            nc.vector.memset(mask[:], 0.0)

            qT = st.tile([64, ICHUNK], F32)
            kT = st.tile([64, N], F32)
            Vraw = st.tile([128, 1024], F32)
            Vn = st.tile([128, 1024], F32)
            Vng = st.tile([128, 1024], F32)
            BD = st.tile([64, NT * 128], F32)
            vs_sb = st.tile([1, 1024], F32)
            Vsum = st.tile([1, 64], F32)

            # ---------------- prologue ----------------
            with tc.tile_pool(name="pps", bufs=1, space="PSUM") as pps:
                qps = pps.tile([64, ICHUNK], F32, tag="q")
                nc.tensor.matmul(qps[:], wqT[:], xT[:], start=True, stop=True)
                nc.scalar.copy(out=qT[:], in_=qps[:])
                for q4 in range(4):
                    kps = pps.tile([64, 512], F32, tag="k")
                    nc.tensor.matmul(kps[:], wkT[:],
                                     ctxT[:, q4 * 512:(q4 + 1) * 512],
                                     start=True, stop=True)
                    nc.scalar.copy(out=kT[:, q4 * 512:(q4 + 1) * 512], in_=kps[:])
                for c in range(16):
                    vps = pps.tile([128, 64], F32, tag="v")
                    nc.tensor.matmul(vps[:], ctxT[:, c * 128:(c + 1) * 128],
                                     wvT[:], start=True, stop=True)
                    nc.vector.tensor_copy(out=Vraw[:, c * 64:(c + 1) * 64],
                                          in_=vps[:])

                # per-head LayerNorm of v over d (groups of 8 in free dim)
                MU8 = st.tile([128, 128], F32)
                S2 = st.tile([128, 128], F32)
                Vsq = st.tile([128, 1024], F32)
                v4 = Vraw[:].rearrange("p (c h d) -> p c h d", h=8, d=8)
                nc.vector.tensor_reduce(out=MU8[:], in_=v4, axis=AX, op=OP.add)
                nc.vector.tensor_mul(out=Vsq[:], in0=Vraw[:], in1=Vraw[:])
                nc.vector.tensor_reduce(
                    out=S2[:], in_=Vsq[:].rearrange("p (c h d) -> p c h d", h=8, d=8),
                    axis=AX, op=OP.add)
                mu = st.tile([128, 128], F32)
                nc.vector.tensor_scalar_mul(out=mu[:], in0=MU8[:], scalar1=0.125)
                musq = st.tile([128, 128], F32)
                nc.vector.tensor_mul(out=musq[:], in0=mu[:], in1=mu[:])
                varv = st.tile([128, 128], F32)
                nc.vector.tensor_scalar_mul(out=varv[:], in0=S2[:],
                                            scalar1=0.125)
                nc.vector.tensor_sub(out=varv[:], in0=varv[:], in1=musq[:])
                nc.vector.tensor_scalar_add(out=varv[:], in0=varv[:],
                                            scalar1=float(EPS))
                lnv = st.tile([128, 128], F32)
                nc.scalar.activation(out=lnv[:], in_=varv[:], func=AF.Ln)
                rv = st.tile([128, 128], F32)
                nc.scalar.activation(out=rv[:], in_=lnv[:], func=AF.Exp,
                                     scale=-0.5)
                muv = mu[:].rearrange("p (c h) -> p c h", h=8)
                mub = bcast_ap(muv, [muv.ap[0], muv.ap[1], muv.ap[2], [0, 8]])
                rvv = rv[:].rearrange("p (c h) -> p c h", h=8)
                rvb = bcast_ap(rvv, [rvv.ap[0], rvv.ap[1], rvv.ap[2], [0, 8]])
                nc.vector.tensor_sub(out=v4, in0=v4, in1=mub)
                nc.vector.tensor_mul(out=v4, in0=v4, in1=rvb)
                v3 = Vraw[:].rearrange("p (c hd) -> p c hd", hd=64)
                gva = gvf[:]
                gvb = bcast_ap(gva, [gva.ap[0], [0, 16], [1, 64]])
                bva = bvf[:]
                bvb = bcast_ap(bva, [bva.ap[0], [0, 16], [1, 64]])
                nc.vector.tensor_mul(out=Vn[:].rearrange("p (c hd) -> p c hd", hd=64),
                                     in0=v3, in1=gvb)
                nc.vector.tensor_add(out=Vn[:].rearrange("p (c hd) -> p c hd", hd=64),
                                     in0=Vn[:].rearrange("p (c hd) -> p c hd", hd=64),
                                     in1=bvb)
                if use_beta:
                    for hf in range(2):
                        vsps = pps.tile([1, 512], F32, tag="vs")
                        nc.tensor.matmul(vsps[:], ones[:],
                                         Vn[:, hf * 512:(hf + 1) * 512],
                                         start=True, stop=True)
                        nc.vector.tensor_copy(
                            out=vs_sb[:, hf * 512:(hf + 1) * 512], in_=vsps[:])
                    vsv = vs_sb[:]
                    nc.vector.tensor_reduce(
                        out=Vsum[:],
                        in_=bcast_ap(vsv, [vsv.ap[0], [1, 64], [64, 16]]),
                        axis=AX, op=OP.add)
                if use_gamma:
                    gta = gtf[:]
                    gtb = bcast_ap(gta, [gta.ap[0], [0, 16], [1, 64]])
                    nc.vector.tensor_mul(
                        out=Vng[:].rearrange("p (c hd) -> p c hd", hd=64),
                        in0=Vn[:].rearrange("p (c hd) -> p c hd", hd=64), in1=gtb)
                    AVrhs = Vng
                else:
                    AVrhs = Vn

                # rebuild mask properly: zero, then write 8 diagonal blocks
                for g in range(8):
                    nc.sync.dma_start(
                        out=mask[g * 16:(g + 1) * 16, g * 8:(g + 1) * 8],
                        in_=bcast_ap(mrow[:, g, :], [[0, 16], [1, 8]]),
                    )

                # block-diagonal packed q: BD[(h,d), (t, h, i16)] = qT[(h,d), (t,i)]
                nc.vector.memset(BD[:], 0.0)
                BD3 = BD[:].rearrange("p (t c) -> p t c", c=128)
                qT3 = qT[:].rearrange("p (t i) -> p t i", i=16)
                for h in range(8):
                    nc.sync.dma_start(
                        out=BD3[h * 8:(h + 1) * 8, :, h * 16:(h + 1) * 16],
                        in_=qT3[h * 8:(h + 1) * 8, :, :])

            # ---------------- main loop ----------------
            with tc.tile_pool(name="sps", bufs=1, space="PSUM") as sps, \
                 tc.tile_pool(name="aps", bufs=5, space="PSUM") as aps, \
                 tc.tile_pool(name="avps", bufs=1, space="PSUM") as avps, \
                 tc.tile_pool(name="le", bufs=2) as le, \
                 tc.tile_pool(name="lsq", bufs=2) as lsq, \
                 tc.tile_pool(name="lp", bufs=2) as lp, \
                 tc.tile_pool(name="lt", bufs=2) as lt, \
                 tc.tile_pool(name="lsm", bufs=4) as lsm, \
                 tc.tile_pool(name="lout", bufs=3) as lout:
                for t in range(NT):
                    bd_t = BD[:, t * 128:(t + 1) * 128]
                    E = le.tile([128, N], F32, tag="E")
                    den4 = lsm.tile([128, 2], F32, tag="den4")
                    for q2 in range(2):
                        s_t = sps.tile([128, 1024], F32, tag="s")
                        for qh in range(2):
                            nc.tensor.matmul(
                                s_t[:, qh * 512:(qh + 1) * 512], bd_t,
                                kT[:, (q2 * 2 + qh) * 512:(q2 * 2 + qh + 1) * 512],
                                start=True, stop=True)
                        nc.scalar.activation(
                            out=E[:, q2 * 1024:(q2 + 1) * 1024], in_=s_t[:],
                            func=AF.Exp, accum_out=den4[:, q2:q2 + 1])
                    den = lsm.tile([128, 1], F32, tag="den")
                    nc.vector.tensor_reduce(out=den[:], in_=den4[:], axis=AX,
                                            op=OP.add)
                    rden = lsm.tile([128, 1], F32, tag="rden")
                    scr = lsm.tile([128, 1], F32, tag="scr")
                    nc.vector.reciprocal_approx_accurate(out=rden[:], in_=den[:],
                                                         scratch=scr[:])
                    tw = lt.tile([128, 128], F32, tag="tw")
                    nc.vector.tensor_scalar_mul(out=tw[:], in0=WD[:],
                                                scalar1=rden[:])
                    SQ = lsq.tile([128, N], F32, tag="SQ")
                    a_tiles = []
                    for b4 in range(4):
                        a_t = aps.tile([128, 512], F32, tag="a")
                        a_tiles.append(a_t)
                        for cl in range(4):
                            c = b4 * 4 + cl
                            nc.tensor.matmul(a_t[:, cl * 128:(cl + 1) * 128],
                                             E[:, c * 128:(c + 1) * 128], tw[:],
                                             start=True, stop=True)
                        if b4 < 2:
                            nc.scalar.square(out=SQ[:, b4 * 512:(b4 + 1) * 512],
                                             in_=a_t[:])
                        else:
                            acp = lsq.tile([128, 512], F32, tag="ACP")
                            nc.vector.tensor_copy(out=acp[:], in_=a_t[:])
                            nc.vector.tensor_mul(
                                out=SQ[:, b4 * 512:(b4 + 1) * 512],
                                in0=acp[:], in1=acp[:])
                    # sum over g (tree adds, strided): SQ free = (c16, g8, i16)
                    T64 = lsq.tile([128, 1024], F32, tag="T64")
                    T32 = lsq.tile([128, 512], F32, tag="T32")
                    M2 = lsm.tile([128, 256], F32, tag="M2")
                    sqv = SQ[:].rearrange("p (c x) -> p c x", x=128)
                    t64v = T64[:].rearrange("p (c x) -> p c x", x=64)
                    nc.gpsimd.tensor_add(out=t64v, in0=sqv[:, :, 0:64],
                                         in1=sqv[:, :, 64:128])
                    t32v = T32[:].rearrange("p (c x) -> p c x", x=32)
                    nc.gpsimd.tensor_add(out=t32v, in0=t64v[:, :, 0:32],
                                         in1=t64v[:, :, 32:64])
                    m2v = M2[:].rearrange("p (c x) -> p c x", x=16)
                    nc.gpsimd.tensor_add(out=m2v, in0=t32v[:, :, 0:16],
                                         in1=t32v[:, :, 16:32])
                    VP = lsm.tile([128, 256], F32, tag="VP")
                    nc.vector.tensor_scalar(out=VP[:], in0=M2[:], scalar1=0.125,
                                            scalar2=float(EPS), op0=OP.mult,
                                            op1=OP.add)
                    LT = lsm.tile([128, 256], F32, tag="LT")
                    nc.scalar.activation(out=LT[:], in_=VP[:], func=AF.Ln)
                    R = lsm.tile([128, 256], F32, tag="R")
                    nc.scalar.activation(out=R[:], in_=LT[:], func=AF.Exp,
                                         scale=-0.5)
                    P = lp.tile([128, N], F32, tag="P")
                    Rv = R[:].rearrange("p (c i) -> p c i", i=16)
                    for b4 in range(4):
                        rb_base = Rv[:, b4 * 4:(b4 + 1) * 4, :]
                        rb = bcast_ap(rb_base, [rb_base.ap[0], rb_base.ap[1],
                                                [0, 8], rb_base.ap[2]])
                        av_in = a_tiles[b4][:].rearrange(
                            "p (c g i) -> p c g i", g=8, i=16)
                        pv = P[:, b4 * 512:(b4 + 1) * 512].rearrange(
                            "p (c g i) -> p c g i", g=8, i=16)
                        nc.vector.tensor_mul(out=pv, in0=av_in, in1=rb)
                    av = avps.tile([128, 64], F32, tag="av")
                    for c in range(16):
                        nc.tensor.matmul(av[:], P[:, c * 128:(c + 1) * 128],
                                         AVrhs[:, c * 64:(c + 1) * 64],
                                         start=(c == 0),
                                         stop=(c == 15 and not use_beta))
                    if use_beta:
                        nc.tensor.matmul(av[:], beta[:], Vsum[:],
                                         start=False, stop=True)
                    EX = lout.tile([128, 64], F32, tag="EX")
                    nc.vector.tensor_mul(out=EX[:], in0=av[:], in1=mask[:])
                    RES = lout.tile([128, 8], F32, tag="RES")
                    nc.vector.tensor_reduce(
                        out=RES[:],
                        in_=EX[:].rearrange("p (h d) -> p d h", h=8),
                        axis=AX, op=OP.add)
                    nc.sync.dma_start(out=d_out[t * 128:(t + 1) * 128, :],
                                      in_=RES[:])
    nc.compile()
    return nc


def _get_module(use_beta, use_gamma):
    key = (use_beta, use_gamma)
    if key not in _CACHE:
        _CACHE[key] = _build(use_beta, use_gamma)
    return _CACHE[key]


def kernel(x, context, Wq, Wkv, g_v, b_v, W_talk, g_t, b_t, **_unused):
    from concourse.bass_utils import run_bass_kernel_spmd

    x = np.asarray(x, np.float32)
    context = np.asarray(context, np.float32)
    Wq = np.asarray(Wq, np.float32)
    Wkv = np.asarray(Wkv, np.float32)
    g_v = np.asarray(g_v, np.float32)
    b_v = np.asarray(b_v, np.float32)
    W_talk = np.asarray(W_talk, np.float32)
    g_t = np.asarray(g_t, np.float32)
    b_t = np.asarray(b_t, np.float32)

    use_beta = bool(np.any(b_t != 0.0))
    use_gamma = bool(np.any(g_t != 1.0))
    nc = _get_module(use_beta, use_gamma)

    wqT = np.ascontiguousarray(Wq.T) * np.float32(DIM ** -0.5)
    wkT = np.ascontiguousarray(Wkv[:DIM, :].T)
    wvT = np.ascontiguousarray(Wkv[DIM:, :].T)
    Wc = W_talk - W_talk.mean(axis=0, keepdims=True)
    WD = np.zeros((8, 16, 8, 16), np.float32)
    for i in range(16):
        WD[:, i, :, i] = Wc.T          # WD[h,i,g,i] = Wc[g,h]
    WD = np.ascontiguousarray(WD.reshape(128, 128))
    beta = np.ascontiguousarray(np.repeat(b_t, 16)[None, :])
    gtf = np.ascontiguousarray(np.repeat(g_t, 8)[None, :])
    gvf = np.ascontiguousarray(np.tile(g_v, 8)[None, :])
    bvf = np.ascontiguousarray(np.tile(b_v, 8)[None, :])
    mrow = np.ones((1, 64), np.float32)
    ones = np.ones((128, 1), np.float32)

    in_maps = []
    for c in range(NCORES):
        b = c // 4
        i0 = (c % 4) * ICHUNK
        in_maps.append({
            "xT": np.ascontiguousarray(x[b, i0:i0 + ICHUNK, :].T),
            "ctxT": np.ascontiguousarray(context[b].T),
            "wqT": wqT, "wkT": wkT, "wvT": wvT, "WD": WD, "beta": beta,
            "gtf": gtf, "gvf": gvf, "bvf": bvf, "mask": mrow, "ones": ones,
        })
    trace_dir = globals().get("TRACE_TMPDIR")
    if trace_dir:
        res = run_bass_kernel_spmd(nc, in_maps, list(range(NCORES)),
                                   trace=True, tmpdir=trace_dir)
        globals()["LAST_EXEC_NS"] = res.exec_time_ns
    else:
        res = run_bass_kernel_spmd(nc, in_maps, list(range(NCORES)))
    out = np.empty((B, 2048, DIM), np.float32)
    for c in range(NCORES):
        b = c // 4
        i0 = (c % 4) * ICHUNK
        o = res.results[c]["out"].reshape(NT, 8, 16, 8)
        out[b, i0:i0 + ICHUNK, :] = (
            o.transpose(0, 2, 1, 3).reshape(ICHUNK, DIM))
    return out


# revision 18
# speedup vs baseline: 1.6046x; 1.6046x over previous
"""Trainium2 Bass kernel for nn_CrossAttention_56092272886201.

Talking-heads cross-attention, b=2, n=m=2048, dim=64, heads=8, dh=dv=8.
Sharding: 8 cores = (batch 2) x (query-chunks of 512). Each core is fully
independent (talking-heads mixes the heads axis, which stays on-core; the
query axis i is sharded), so no collectives are needed.

Per-core layout (i-tile = 16 queries x 8 heads = 128 partitions):
  S[(h,i), j]   = QK^T via block-diagonal packed q (one matmul, K=64)
  E = exp(S)    ACT, fused row-sum -> softmax denominator
  TW            = WD * (1/denom) per partition  (denom + talking-heads fold)
  A^T[j,(g,i)]  = talk matmul, lhsT=E-chunk (output already j-partitioned)
  m2            = sum_g A^2 (squares split ACT/DVE, strided tree adds on GPSIMD)
  r             = exp(-0.5*ln(m2/8+eps))   (rsqrt; ln+exp share one ACT table set)
  P = A*r       broadcast multiply
  out           = P @ V_ln (gamma_t folded into V, beta_t via K=1 matmul)
W_talk is centered host-side over g so the heads-LayerNorm mean is exactly 0.
"""

import numpy as np

DIM = 64
HEADS = 8
N = 2048
B = 2
NCORES = 8
ICHUNK = 512          # queries per core
NT = 32               # i-tiles per core (16 queries each)
EPS = 1e-5

_CACHE = {}
# bf16 stationary/moving operands for the talk and AV matmul chains (QK stays
# fp32, all PSUM accumulation and softmax/LN statistics stay fp32). fp32
# matmuls double-pump on TRN2 (2 HW ops + 2x LDWEIGHTS); bf16 halves the
# PE-array occupancy of the 33-matmul-per-tile talk/AV chains.
USE_BF16 = True


def _build(use_beta, use_gamma):
    import concourse.bacc as bacc
    import concourse.tile as tile
    from concourse import mybir

    F32 = mybir.dt.float32
    MMDT = mybir.dt.bfloat16 if USE_BF16 else mybir.dt.float32
    AX = mybir.AxisListType.X
    OP = mybir.AluOpType
    AF = mybir.ActivationFunctionType

    nc = bacc.Bacc()
    d_xT = nc.declare_dram_parameter("xT", [64, ICHUNK], F32, isOutput=False)
    d_ctxT = nc.declare_dram_parameter("ctxT", [64, N], F32, isOutput=False)
    d_wqT = nc.declare_dram_parameter("wqT", [64, 64], F32, isOutput=False)
    d_wkT = nc.declare_dram_parameter("wkT", [64, 64], F32, isOutput=False)
    d_wvT = nc.declare_dram_parameter("wvT", [64, 64], F32, isOutput=False)
    d_WD = nc.declare_dram_parameter("WD", [128, 128], F32, isOutput=False)
    d_beta = nc.declare_dram_parameter("beta", [1, 128], F32, isOutput=False)
    d_gtf = nc.declare_dram_parameter("gtf", [1, 64], F32, isOutput=False)
    d_gvf = nc.declare_dram_parameter("gvf", [1, 64], F32, isOutput=False)
    d_bvf = nc.declare_dram_parameter("bvf", [1, 64], F32, isOutput=False)
    d_mask = nc.declare_dram_parameter("mask", [1, 64], F32, isOutput=False)
    d_ones = nc.declare_dram_parameter("ones", [128, 1], F32, isOutput=False)
    d_out = nc.declare_dram_parameter("out", [NT * 128, 8], F32, isOutput=True)

    import concourse.bass as bass

    def bcast_ap(ap, levels):
        return bass.AP(tensor=ap.tensor, offset=ap.offset, ap=levels)

    with tile.TileContext(nc) as tc:
        with tc.tile_pool(name="statics", bufs=1) as st:
            xT = st.tile([64, ICHUNK], F32)
            ctxT = st.tile([64, N], F32)
            wqT = st.tile([64, 64], F32)
            wkT = st.tile([64, 64], F32)
            wvT = st.tile([64, 64], F32)
            WD = st.tile([128, 128], F32)
            beta = st.tile([1, 128], F32)
            gtf = st.tile([128, 64], F32)
            gvf = st.tile([128, 64], F32)
            bvf = st.tile([128, 64], F32)
            # head-select mask replicated to all partitions via DMA broadcast
            mask = st.tile([128, 64], F32)
            ones = st.tile([128, 1], F32)
            for sb, dr in ((xT, d_xT), (ctxT, d_ctxT), (wqT, d_wqT),
                           (wkT, d_wkT), (wvT, d_wvT), (WD, d_WD),
                           (beta, d_beta), (ones, d_ones)):
                nc.sync.dma_start(out=sb[:], in_=dr[:])
            # replicate [1,64] host rows across all 128 partitions
            for sb, dr in ((gtf, d_gtf), (gvf, d_gvf), (bvf, d_bvf)):
                nc.sync.dma_start(
                    out=sb[:], in_=bcast_ap(dr[:], [[0, 128], [1, 64]]))
            # mask[(g,i),(h,d)] = (h==g): zero then fill 8 diagonal blocks
            # from the [1,64] host ones-row via broadcast DMAs.
            mrow = d_mask.rearrange("o (h d) -> o h d", h=8)
            nc.vector.memset(mask[:], 0.0)

            qT = st.tile([64, ICHUNK], F32)
            kT = st.tile([64, N], F32)
            Vraw = st.tile([128, 1024], F32)
            Vn = st.tile([128, 1024], F32)
            Vng = st.tile([128, 1024], F32)
            BD = st.tile([64, NT * 128], F32)
            vs_sb = st.tile([1, 1024], F32)
            Vsum = st.tile([1, 64], F32)

            # ---------------- prologue ----------------
            with tc.tile_pool(name="pps", bufs=1, space="PSUM") as pps:
                qps = pps.tile([64, ICHUNK], F32, tag="q")
                nc.tensor.matmul(qps[:], wqT[:], xT[:], start=True, stop=True)
                nc.scalar.copy(out=qT[:], in_=qps[:])
                for q4 in range(4):
                    kps = pps.tile([64, 512], F32, tag="k")
                    nc.tensor.matmul(kps[:], wkT[:],
                                     ctxT[:, q4 * 512:(q4 + 1) * 512],
                                     start=True, stop=True)
                    nc.scalar.copy(out=kT[:, q4 * 512:(q4 + 1) * 512], in_=kps[:])
                for c in range(16):
                    vps = pps.tile([128, 64], F32, tag="v")
                    nc.tensor.matmul(vps[:], ctxT[:, c * 128:(c + 1) * 128],
                                     wvT[:], start=True, stop=True)
                    nc.vector.tensor_copy(out=Vraw[:, c * 64:(c + 1) * 64],
                                          in_=vps[:])

                # per-head LayerNorm of v over d (groups of 8 in free dim)
                MU8 = st.tile([128, 128], F32)
                S2 = st.tile([128, 128], F32)
                Vsq = st.tile([128, 1024], F32)
                v4 = Vraw[:].rearrange("p (c h d) -> p c h d", h=8, d=8)
                nc.vector.tensor_reduce(out=MU8[:], in_=v4, axis=AX, op=OP.add)
                nc.vector.tensor_mul(out=Vsq[:], in0=Vraw[:], in1=Vraw[:])
                nc.vector.tensor_reduce(
                    out=S2[:], in_=Vsq[:].rearrange("p (c h d) -> p c h d", h=8, d=8),
                    axis=AX, op=OP.add)
                mu = st.tile([128, 128], F32)
                nc.vector.tensor_scalar_mul(out=mu[:], in0=MU8[:], scalar1=0.125)
                musq = st.tile([128, 128], F32)
                nc.vector.tensor_mul(out=musq[:], in0=mu[:], in1=mu[:])
                varv = st.tile([128, 128], F32)
                nc.vector.tensor_scalar_mul(out=varv[:], in0=S2[:],
                                            scalar1=0.125)
                nc.vector.tensor_sub(out=varv[:], in0=varv[:], in1=musq[:])
                nc.vector.tensor_scalar_add(out=varv[:], in0=varv[:],
                                            scalar1=float(EPS))
                lnv = st.tile([128, 128], F32)
                nc.scalar.activation(out=lnv[:], in_=varv[:], func=AF.Ln)
                rv = st.tile([128, 128], F32)
                nc.scalar.activation(out=rv[:], in_=lnv[:], func=AF.Exp,
                                     scale=-0.5)
                muv = mu[:].rearrange("p (c h) -> p c h", h=8)
                mub = bcast_ap(muv, [muv.ap[0], muv.ap[1], muv.ap[2], [0, 8]])
                rvv = rv[:].rearrange("p (c h) -> p c h", h=8)
                rvb = bcast_ap(rvv, [rvv.ap[0], rvv.ap[1], rvv.ap[2], [0, 8]])
                nc.vector.tensor_sub(out=v4, in0=v4, in1=mub)
                nc.vector.tensor_mul(out=v4, in0=v4, in1=rvb)
                v3 = Vraw[:].rearrange("p (c hd) -> p c hd", hd=64)
                gva = gvf[:]
                gvb = bcast_ap(gva, [gva.ap[0], [0, 16], [1, 64]])
                bva = bvf[:]
                bvb = bcast_ap(bva, [bva.ap[0], [0, 16], [1, 64]])
                nc.vector.tensor_mul(out=Vn[:].rearrange("p (c hd) -> p c hd", hd=64),
                                     in0=v3, in1=gvb)
                nc.vector.tensor_add(out=Vn[:].rearrange("p (c hd) -> p c hd", hd=64),
                                     in0=Vn[:].rearrange("p (c hd) -> p c hd", hd=64),
                                     in1=bvb)
                if use_beta:
                    for hf in range(2):
                        vsps = pps.tile([1, 512], F32, tag="vs")
                        nc.tensor.matmul(vsps[:], ones[:],
                                         Vn[:, hf * 512:(hf + 1) * 512],
                                         start=True, stop=True)
                        nc.vector.tensor_copy(
                            out=vs_sb[:, hf * 512:(hf + 1) * 512], in_=vsps[:])
                    vsv = vs_sb[:]
                    nc.vector.tensor_reduce(
                        out=Vsum[:],
                        in_=bcast_ap(vsv, [vsv.ap[0], [1, 64], [64, 16]]),
                        axis=AX, op=OP.add)
                if use_gamma:
                    gta = gtf[:]
                    gtb = bcast_ap(gta, [gta.ap[0], [0, 16], [1, 64]])
                    nc.vector.tensor_mul(
                        out=Vng[:].rearrange("p (c hd) -> p c hd", hd=64),
                        in0=Vn[:].rearrange("p (c hd) -> p c hd", hd=64), in1=gtb)
                    AVrhs = Vng
                else:
                    AVrhs = Vn
                Vng16 = st.tile([128, 1024], MMDT)
                nc.vector.tensor_copy(out=Vng16[:], in_=AVrhs[:])
                AVrhs = Vng16

                # rebuild mask properly: zero, then write 8 diagonal blocks
                for g in range(8):
                    nc.sync.dma_start(
                        out=mask[g * 16:(g + 1) * 16, g * 8:(g + 1) * 8],
                        in_=bcast_ap(mrow[:, g, :], [[0, 16], [1, 8]]),
                    )

                # block-diagonal packed q: BD[(h,d), (t, h, i16)] = qT[(h,d), (t,i)]
                nc.vector.memset(BD[:], 0.0)
                BD3 = BD[:].rearrange("p (t c) -> p t c", c=128)
                qT3 = qT[:].rearrange("p (t i) -> p t i", i=16)
                for h in range(8):
                    nc.sync.dma_start(
                        out=BD3[h * 8:(h + 1) * 8, :, h * 16:(h + 1) * 16],
                        in_=qT3[h * 8:(h + 1) * 8, :, :])

            # ---------------- main loop ----------------
            with tc.tile_pool(name="sps", bufs=1, space="PSUM") as sps, \
                 tc.tile_pool(name="aps", bufs=5, space="PSUM") as aps, \
                 tc.tile_pool(name="avps", bufs=1, space="PSUM") as avps, \
                 tc.tile_pool(name="le", bufs=2) as le, \
                 tc.tile_pool(name="lsq", bufs=2) as lsq, \
                 tc.tile_pool(name="lp", bufs=2) as lp, \
                 tc.tile_pool(name="lt", bufs=2) as lt, \
                 tc.tile_pool(name="lsm", bufs=4) as lsm, \
                 tc.tile_pool(name="lout", bufs=3) as lout:
                for t in range(NT):
                    bd_t = BD[:, t * 128:(t + 1) * 128]
                    E = le.tile([128, N], MMDT, tag="E")
                    den4 = lsm.tile([128, 2], F32, tag="den4")
                    for q2 in range(2):
                        s_t = sps.tile([128, 1024], F32, tag="s")
                        for qh in range(2):
                            nc.tensor.matmul(
                                s_t[:, qh * 512:(qh + 1) * 512], bd_t,
                                kT[:, (q2 * 2 + qh) * 512:(q2 * 2 + qh + 1) * 512],
                                start=True, stop=True)
                        nc.scalar.activation(
                            out=E[:, q2 * 1024:(q2 + 1) * 1024], in_=s_t[:],
                            func=AF.Exp, accum_out=den4[:, q2:q2 + 1])
                    den = lsm.tile([128, 1], F32, tag="den")
                    nc.vector.tensor_reduce(out=den[:], in_=den4[:], axis=AX,
                                            op=OP.add)
                    rden = lsm.tile([128, 1], F32, tag="rden")
                    scr = lsm.tile([128, 1], F32, tag="scr")
                    nc.vector.reciprocal_approx_accurate(out=rden[:], in_=den[:],
                                                         scratch=scr[:])
                    tw = lt.tile([128, 128], MMDT, tag="tw")
                    nc.vector.tensor_scalar_mul(out=tw[:], in0=WD[:],
                                                scalar1=rden[:])
                    SQ = lsq.tile([128, N], F32, tag="SQ")
                    a_tiles = []
                    for b4 in range(4):
                        a_t = aps.tile([128, 512], F32, tag="a")
                        a_tiles.append(a_t)
                        for cl in range(4):
                            c = b4 * 4 + cl
                            nc.tensor.matmul(a_t[:, cl * 128:(cl + 1) * 128],
                                             E[:, c * 128:(c + 1) * 128], tw[:],
                                             start=True, stop=True)
                        if b4 < 2:
                            nc.scalar.square(out=SQ[:, b4 * 512:(b4 + 1) * 512],
                                             in_=a_t[:])
                        else:
                            acp = lsq.tile([128, 512], F32, tag="ACP")
                            nc.vector.tensor_copy(out=acp[:], in_=a_t[:])
                            nc.vector.tensor_mul(
                                out=SQ[:, b4 * 512:(b4 + 1) * 512],
                                in0=acp[:], in1=acp[:])
                    # sum over g (tree adds, strided): SQ free = (c16, g8, i16)
                    T64 = lsq.tile([128, 1024], F32, tag="T64")
                    T32 = lsq.tile([128, 512], F32, tag="T32")
                    M2 = lsm.tile([128, 256], F32, tag="M2")
                    sqv = SQ[:].rearrange("p (c x) -> p c x", x=128)
                    t64v = T64[:].rearrange("p (c x) -> p c x", x=64)
                    nc.gpsimd.tensor_add(out=t64v, in0=sqv[:, :, 0:64],
                                         in1=sqv[:, :, 64:128])
                    t32v = T32[:].rearrange("p (c x) -> p c x", x=32)
                    nc.gpsimd.tensor_add(out=t32v, in0=t64v[:, :, 0:32],
                                         in1=t64v[:, :, 32:64])
                    m2v = M2[:].rearrange("p (c x) -> p c x", x=16)
                    nc.gpsimd.tensor_add(out=m2v, in0=t32v[:, :, 0:16],
                                         in1=t32v[:, :, 16:32])
                    VP = lsm.tile([128, 256], F32, tag="VP")
                    nc.vector.tensor_scalar(out=VP[:], in0=M2[:], scalar1=0.125,
                                            scalar2=float(EPS), op0=OP.mult,
                                            op1=OP.add)
                    LT = lsm.tile([128, 256], F32, tag="LT")
                    nc.scalar.activation(out=LT[:], in_=VP[:], func=AF.Ln)
                    R = lsm.tile([128, 256], F32, tag="R")
                    nc.scalar.activation(out=R[:], in_=LT[:], func=AF.Exp,
                                         scale=-0.5)
                    P = lp.tile([128, N], MMDT, tag="P")
                    Rv = R[:].rearrange("p (c i) -> p c i", i=16)
                    for b4 in range(4):
                        rb_base = Rv[:, b4 * 4:(b4 + 1) * 4, :]
                        rb = bcast_ap(rb_base, [rb_base.ap[0], rb_base.ap[1],
                                                [0, 8], rb_base.ap[2]])
                        av_in = a_tiles[b4][:].rearrange(
                            "p (c g i) -> p c g i", g=8, i=16)
                        pv = P[:, b4 * 512:(b4 + 1) * 512].rearrange(
                            "p (c g i) -> p c g i", g=8, i=16)
                        nc.vector.tensor_mul(out=pv, in0=av_in, in1=rb)
                    av = avps.tile([128, 64], F32, tag="av")
                    for c in range(16):
                        nc.tensor.matmul(av[:], P[:, c * 128:(c + 1) * 128],
                                         AVrhs[:, c * 64:(c + 1) * 64],
                                         start=(c == 0),
                                         stop=(c == 15 and not use_beta))
                    if use_beta:
                        nc.tensor.matmul(av[:], beta[:], Vsum[:],
                                         start=False, stop=True)
                    EX = lout.tile([128, 64], F32, tag="EX")
                    nc.vector.tensor_mul(out=EX[:], in0=av[:], in1=mask[:])
                    RES = lout.tile([128, 8], F32, tag="RES")
                    nc.vector.tensor_reduce(
                        out=RES[:],
                        in_=EX[:].rearrange("p (h d) -> p d h", h=8),
                        axis=AX, op=OP.add)
                    nc.sync.dma_start(out=d_out[t * 128:(t + 1) * 128, :],
                                      in_=RES[:])
    nc.compile()
    return nc


def _get_module(use_beta, use_gamma):
    key = (use_beta, use_gamma)
    if key not in _CACHE:
        _CACHE[key] = _build(use_beta, use_gamma)
    return _CACHE[key]


def kernel(x, context, Wq, Wkv, g_v, b_v, W_talk, g_t, b_t, **_unused):
    from concourse.bass_utils import run_bass_kernel_spmd

    x = np.asarray(x, np.float32)
    context = np.asarray(context, np.float32)
    Wq = np.asarray(Wq, np.float32)
    Wkv = np.asarray(Wkv, np.float32)
    g_v = np.asarray(g_v, np.float32)
    b_v = np.asarray(b_v, np.float32)
    W_talk = np.asarray(W_talk, np.float32)
    g_t = np.asarray(g_t, np.float32)
    b_t = np.asarray(b_t, np.float32)

    use_beta = bool(np.any(b_t != 0.0))
    use_gamma = bool(np.any(g_t != 1.0))
    nc = _get_module(use_beta, use_gamma)

    wqT = np.ascontiguousarray(Wq.T) * np.float32(DIM ** -0.5)
    wkT = np.ascontiguousarray(Wkv[:DIM, :].T)
    wvT = np.ascontiguousarray(Wkv[DIM:, :].T)
    Wc = W_talk - W_talk.mean(axis=0, keepdims=True)
    WD = np.zeros((8, 16, 8, 16), np.float32)
    for i in range(16):
        WD[:, i, :, i] = Wc.T          # WD[h,i,g,i] = Wc[g,h]
    WD = np.ascontiguousarray(WD.reshape(128, 128))
    beta = np.ascontiguousarray(np.repeat(b_t, 16)[None, :])
    gtf = np.ascontiguousarray(np.repeat(g_t, 8)[None, :])
    gvf = np.ascontiguousarray(np.tile(g_v, 8)[None, :])
    bvf = np.ascontiguousarray(np.tile(b_v, 8)[None, :])
    mrow = np.ones((1, 64), np.float32)
    ones = np.ones((128, 1), np.float32)

    in_maps = []
    for c in range(NCORES):
        b = c // 4
        i0 = (c % 4) * ICHUNK
        in_maps.append({
            "xT": np.ascontiguousarray(x[b, i0:i0 + ICHUNK, :].T),
            "ctxT": np.ascontiguousarray(context[b].T),
            "wqT": wqT, "wkT": wkT, "wvT": wvT, "WD": WD, "beta": beta,
            "gtf": gtf, "gvf": gvf, "bvf": bvf, "mask": mrow, "ones": ones,
        })
    trace_dir = globals().get("TRACE_TMPDIR")
    if trace_dir:
        res = run_bass_kernel_spmd(nc, in_maps, list(range(NCORES)),
                                   trace=True, tmpdir=trace_dir)
        globals()["LAST_EXEC_NS"] = res.exec_time_ns
    else:
        res = run_bass_kernel_spmd(nc, in_maps, list(range(NCORES)))
    out = np.empty((B, 2048, DIM), np.float32)
    for c in range(NCORES):
        b = c // 4
        i0 = (c % 4) * ICHUNK
        o = res.results[c]["out"].reshape(NT, 8, 16, 8)
        out[b, i0:i0 + ICHUNK, :] = (
            o.transpose(0, 2, 1, 3).reshape(ICHUNK, DIM))
    return out
